# revision 5
# baseline (speedup 1.0000x reference)
"""Trainium2 Bass kernel for nn_NR_GraphAttention (2-layer relational graph
attention, N=50000 nodes, T=400000 edges, D=256, R=1000 relations, 8 cores).

Strategy
--------
Edges are partitioned by destination-node range (6250 nodes per core), so all
segment ops (softmax denominators, scatter sums) are core-local.  Within a
core, edges are grouped into 49 dst tiles of 128 nodes, each padded to CH
128-edge chunks.  Per chunk the scatter/aggregation runs as one-hot f32r
matmuls accumulating into PSUM.  Host-side preprocessing is pure input
indexing: edge sort, padding, and pre-permuted per-edge streams
features[src] / rel_emb[rel] so that layer 1 needs no device-side gathers.
Layer 2 gathers x1[src] via per-chunk indirect DMAs from an AllGathered
feature table.  All math (tanh, normalize, attention, softmax, reflection,
aggregation) happens on device.
"""
import numpy as np

import concourse.bass as bass
import concourse.mybir as mybir
import concourse.tile as tile
from concourse.bass_utils import run_bass_kernel_spmd

# ---------------- problem constants (hardcoded per spec) ----------------
N, R, T, D = 50000, 1000, 400000, 256
DEPTH = 2
W = 8                    # cores
NL = N // W              # 6250 local nodes per core
P = 128
TILES = (NL + P - 1) // P   # 49
NLP = TILES * P             # 6272 padded local nodes
EPS = 1e-12

F32 = mybir.dt.float32
F32R = mybir.dt.float32r
I32 = mybir.dt.int32
AF = mybir.ActivationFunctionType
ALU = mybir.AluOpType
AX = mybir.AxisListType

_counter = [0]


def _split_multi_waits(nc, max_waits=1):
    """walrus in this container rejects >1 sync wait per instruction; hoist
    extra waits onto standalone EventSemaphore instructions (same engine,
    immediately before — semantics preserved, engines execute in order)."""
    n = 0
    for func in nc.m.functions:
        for bb in func.blocks:
            insts = bb.instructions
            if not any(
                i.sync_info is not None and len(i.sync_info.on_wait) > max_waits
                for i in insts
            ):
                continue
            new = []
            for ins in insts:
                si = ins.sync_info
                if si is not None and len(si.on_wait) > max_waits:
                    waits = list(si.on_wait)
                    for wv in waits[:-max_waits]:
                        ev = mybir.InstEventSemaphore(
                            name=f"waitsplit-{_counter[0]}", ins=[], outs=[]
                        )
                        _counter[0] += 1
                        ev.engine = ins.engine
                        ev.sync_info = mybir.SyncInfo(on_wait=[wv], on_update=[])
                        nc.register_instruction(ev, overwrite=True)
                        new.append(ev)
                        n += 1
                    ins.sync_info = mybir.SyncInfo(
                        on_wait=waits[-max_waits:], on_update=list(si.on_update)
                    )
                new.append(ins)
            bb.instructions = new
    return n


def _build(CH):
    """Build the SPMD bass program (identical on all cores)."""
    C = TILES * CH              # chunk columns per core
    EC = C * P                  # padded edges per core
    GD = CH * D                 # group free width
    DD = D + 2                  # x chunk + ones col (fused denom) + zero pad
    GX = CH * DD

    nc = bass.Bass("TRN2", num_devices=W)

    xf_in = nc.dram_tensor("xf_in", [EC, D], F32R, kind="ExternalInput")
    qs_in = nc.dram_tensor("qs_in", [EC, D], F32R, kind="ExternalInput")
    fslab_in = nc.dram_tensor("fslab_in", [NLP, D], F32, kind="ExternalInput")
    dstmod_in = nc.dram_tensor("dstmod_in", [P, C], F32, kind="ExternalInput")
    srcidx_in = nc.dram_tensor("srcidx_in", [P, C], I32, kind="ExternalInput")
    iota_in = nc.dram_tensor("iota_in", [P, P], F32, kind="ExternalInput")
    kbc_in = nc.dram_tensor("kbc_in", [DEPTH, P, GD], F32R, kind="ExternalInput")

    out = nc.dram_tensor("out", [NLP, 3 * D], F32, kind="ExternalOutput")

    x1slab = nc.dram_tensor("x1slab", [NLP, D], F32, kind="Internal")
    x1full = nc.dram_tensor(
        "x1full", [NLP * W, D], F32, kind="Internal", addr_space="Shared"
    )

    with tile.TileContext(nc) as tc:
        with tc.tile_pool(name="const", bufs=1) as cp, \
             tc.tile_pool(name="grp", bufs=3) as gp, \
             tc.tile_pool(name="small", bufs=4) as sp, \
             tc.tile_pool(name="oh", bufs=4) as ohp, \
             tc.tile_pool(name="fin", bufs=3) as fp, \
             tc.tile_pool(name="ps", bufs=2, space="PSUM") as pp:

            # ---- persistent constants ----
            iota_t = cp.tile([P, P], F32)
            nc.sync.dma_start(out=iota_t[:], in_=iota_in[:, :])
            dstmod_t = cp.tile([P, C], F32)
            nc.sync.dma_start(out=dstmod_t[:], in_=dstmod_in[:, :])
            srcidx_t = cp.tile([P, C], I32)
            nc.sync.dma_start(out=srcidx_t[:], in_=srcidx_in[:, :])
            kbc_t = [
                cp.tile([P, GD], F32R, tag=f"kbc{l}", name=f"kbc{l}")
                for l in range(DEPTH)
            ]
            for l in range(DEPTH):
                nc.sync.dma_start(out=kbc_t[l][:], in_=kbc_in[l, :, :])

            # ---- x0 = tanh(features slab) -> out[:, 0:D] ----
            XG = 7   # tiles per x0 group; 49 = 7*7
            for g in range(TILES // XG):
                ft = gp.tile([P, XG * D], F32, tag="x0")
                r0 = g * XG * P
                nc.sync.dma_start(
                    out=ft[:].rearrange("p (c d) -> p c d", d=D),
                    in_=fslab_in[r0:r0 + XG * P, :].rearrange(
                        "(c p) d -> p c d", p=P),
                )
                xt = gp.tile([P, XG * D], F32, tag="x0o")
                nc.scalar.activation(out=xt[:], in_=ft[:], func=AF.Tanh)
                nc.sync.dma_start(
                    out=out[r0:r0 + XG * P, 0:D].rearrange(
                        "(c p) d -> p c d", p=P),
                    in_=xt[:].rearrange("p (c d) -> p c d", d=D),
                )

            # ---- layers ----
            for l in range(DEPTH):
                for t in range(TILES):
                    g0 = t * CH          # first chunk column of this tile
                    e0 = g0 * P          # first padded edge row

                    # per-edge x rows (tanh'd) + ones col: [P, CH*(D+1)]
                    xu = gp.tile([P, GX], F32R, tag="xu")
                    xu3 = xu[:].rearrange("p (c e) -> p c e", e=DD)
                    if l == 0:
                        xg = gp.tile([P, GD], F32R, tag="xg")
                        nc.sync.dma_start(
                            out=xg[:].rearrange("p (c d) -> p c d", d=D),
                            in_=xf_in[e0:e0 + CH * P, :].rearrange(
                                "(c p) d -> p c d", p=P),
                        )
                        nc.scalar.activation(
                            out=xu3[:, :, 0:D],
                            in_=xg[:].rearrange("p (c d) -> p c d", d=D),
                            func=AF.Tanh)
                    else:
                        for c in range(CH):
                            nc.gpsimd.indirect_dma_start(
                                out=xu[:, c * DD:c * DD + D],
                                out_offset=None,
                                in_=x1full[:, :].bitcast(F32R),
                                in_offset=bass.IndirectOffsetOnAxis(
                                    ap=srcidx_t[:, g0 + c:g0 + c + 1], axis=0),
                            )
                    # ones column (fused softmax denominator) + zero pad col
                    nc.vector.tensor_scalar(
                        out=xu3[:, :, D:D + 1], in0=iota_t[:, 0:CH],
                        scalar1=0.0, scalar2=1.0, op0=ALU.mult, op1=ALU.add)
                    nc.vector.tensor_scalar(
                        out=xu3[:, :, D + 1:DD], in0=iota_t[:, 0:CH],
                        scalar1=0.0, scalar2=0.0, op0=ALU.mult, op1=ALU.mult)

                    # per-edge raw rel rows
                    qg = gp.tile([P, GD], F32R, tag="qg")
                    nc.sync.dma_start(
                        out=qg[:].rearrange("p (c d) -> p c d", d=D),
                        in_=qs_in[e0:e0 + CH * P, :].rearrange(
                            "(c p) d -> p c d", p=P),
                    )

                    # d = x . q   per chunk  [P, CH]
                    prod = gp.tile([P, GD], F32, tag="prod")
                    nc.vector.tensor_mul(
                        prod[:].rearrange("p (c d) -> p c d", d=D),
                        xu3[:, :, 0:D],
                        qg[:].rearrange("p (c d) -> p c d", d=D))
                    dcol = sp.tile([P, CH], F32, tag="dcol")
                    nc.vector.tensor_reduce(
                        out=dcol[:],
                        in_=prod[:].rearrange("p (c d) -> p c d", c=CH),
                        axis=AX.X, op=ALU.add)

                    # attraw = q . k_l   per chunk
                    prod2 = gp.tile([P, GD], F32, tag="prod")
                    nc.vector.tensor_mul(prod2[:], qg[:], kbc_t[l][:])
                    attraw = sp.tile([P, CH], F32, tag="attraw")
                    nc.vector.tensor_reduce(
                        out=attraw[:],
                        in_=prod2[:].rearrange("p (c d) -> p c d", c=CH),
                        axis=AX.X, op=ALU.add)

                    # n2 = |q|^2 per chunk via ACT square+accum
                    n2 = sp.tile([P, CH], F32, tag="n2")
                    sqs = gp.tile([P, D], F32, tag="sqs")
                    for c in range(CH):
                        nc.scalar.activation(
                            out=sqs[:], in_=qg[:, c * D:(c + 1) * D],
                            func=AF.Square, accum_out=n2[:, c:c + 1])

                    nrm = sp.tile([P, CH], F32, tag="nrm")
                    nc.scalar.activation(out=nrm[:], in_=n2[:], func=AF.Sqrt)
                    invn = sp.tile([P, CH], F32, tag="invn")
                    nc.vector.reciprocal(invn[:], nrm[:])
                    in2 = sp.tile([P, CH], F32, tag="in2")
                    nc.vector.tensor_mul(in2[:], invn[:], invn[:])
                    # ee = exp(attraw * invn)
                    attn = sp.tile([P, CH], F32, tag="attn")
                    nc.vector.tensor_mul(attn[:], attraw[:], invn[:])
                    ee = sp.tile([P, CH], F32, tag="ee")
                    nc.scalar.activation(out=ee[:], in_=attn[:], func=AF.Exp)
                    # s2 = d * ee * in2   (refl scale before the -2 factor)
                    s1 = sp.tile([P, CH], F32, tag="s1")
                    nc.vector.tensor_mul(s1[:], dcol[:], ee[:])
                    s2 = sp.tile([P, CH], F32, tag="s2")
                    nc.vector.tensor_mul(s2[:], s1[:], in2[:])

                    psum = pp.tile([P, D + 2], F32, space="PSUM", tag="acc")
                    for c in range(CH):
                        ohw = ohp.tile([P, P], F32R, tag="ohw")
                        nc.vector.tensor_scalar(
                            out=ohw[:], in0=iota_t[:],
                            scalar1=dstmod_t[:, g0 + c:g0 + c + 1],
                            scalar2=ee[:, c:c + 1],
                            op0=ALU.is_equal, op1=ALU.mult)
                        ohw2 = ohp.tile([P, P], F32R, tag="ohw2")
                        nc.vector.tensor_scalar(
                            out=ohw2[:], in0=ohw[:].bitcast(F32),
                            scalar1=s2[:, c:c + 1], scalar2=-2.0,
                            op0=ALU.mult, op1=ALU.mult)
                        nc.tensor.matmul(
                            psum[:, 0:DD], lhsT=ohw[:],
                            rhs=xu[:, c * DD:(c + 1) * DD],
                            start=(c == 0), stop=False)
                        nc.tensor.matmul(
                            psum[:, 0:D], lhsT=ohw2[:],
                            rhs=qg[:, c * D:(c + 1) * D],
                            start=False, stop=(c == CH - 1))

                    # finalize tile: x_next = tanh(numer / max(denom, eps))
                    dn = sp.tile([P, 1], F32, tag="dn")
                    nc.vector.tensor_scalar_max(dn[:], psum[:, D:D + 1], EPS)
                    inv = sp.tile([P, 1], F32, tag="inv")
                    nc.vector.reciprocal(inv[:], dn[:])
                    xo = fp.tile([P, D], F32, tag="xo")
                    nc.scalar.activation(
                        out=xo[:], in_=psum[:, 0:D], func=AF.Tanh,
                        scale=inv[:, 0:1])
                    nc.sync.dma_start(
                        out=out[t * P:(t + 1) * P, (l + 1) * D:(l + 2) * D],
                        in_=xo[:])
                    if l == 0:
                        nc.sync.dma_start(
                            out=x1slab[t * P:(t + 1) * P, :], in_=xo[:])

                if l == 0:
                    nc.gpsimd.collective_compute(
                        "AllGather", ALU.bypass,
                        ins=[x1slab[:, :]], outs=[x1full[:, :]],
                        replica_groups=[list(range(W))],
                    )

    _split_multi_waits(nc)
    return nc


def _prepare(inputs):
    """Host preprocessing: pure index manipulation of the inputs."""
    features = np.ascontiguousarray(np.asarray(inputs["features"], dtype=np.float32))
    rel_emb = np.ascontiguousarray(np.asarray(inputs["rel_emb"], dtype=np.float32))
    attn_k = np.asarray(inputs["attn_kernels"], dtype=np.float32)
    r_val = np.asarray(inputs["r_val"])
    adj = np.asarray(inputs["adj"])
    r_index_tri = np.asarray(inputs["r_index_tri"])
    r_index_rel = np.asarray(inputs["r_index_rel"])

    assert np.array_equal(r_index_tri, np.arange(T)), "expected identity triples"
    assert np.all(r_val > 0), "expected positive r_val (scale cancels in normalize)"

    dst = adj[0].astype(np.int64)
    src = adj[1].astype(np.int64)
    rel = r_index_rel.astype(np.int64)

    order = np.argsort(dst, kind="stable")
    dstS, srcS, relS = dst[order], src[order], rel[order]

    # per (core, tile) edge spans
    bounds = np.searchsorted(dstS, np.arange(0, N + 1, P))  # 391+1 tiles of 128
    # global tile g = node//128; core k owns nodes [k*NL, (k+1)*NL) i.e. tiles
    # are split mid-way (NL % P != 0), so compute spans per core-tile directly.
    CH = 0
    spans = []
    for k in range(W):
        row = []
        for t in range(TILES):
            lo = k * NL + t * P
            hi = min(k * NL + (t + 1) * P, (k + 1) * NL)
            a = np.searchsorted(dstS, lo)
            b = np.searchsorted(dstS, hi)
            row.append((a, b, lo))
            CH = max(CH, -(-(b - a) // P))
        spans.append(row)
    C = TILES * CH
    EC = C * P

    per_core = []
    for k in range(W):
        esrc = np.zeros(EC, dtype=np.int64)
        erel = np.zeros(EC, dtype=np.int64)
        edst = np.full(EC, 999.0, dtype=np.float32)
        for t in range(TILES):
            a, b, lo = spans[k][t]
            n = b - a
            o = t * CH * P
            esrc[o:o + n] = srcS[a:b]
            erel[o:o + n] = relS[a:b]
            edst[o:o + n] = (dstS[a:b] - lo).astype(np.float32)
        # lane layout [P, C]: column g, lane p  <-> padded edge g*P + p
        dstmod = edst.reshape(C, P).T.copy()
        # global node id -> row in the AllGathered padded table
        gsrc = (esrc // NL) * NLP + (esrc % NL)
        srcidx = gsrc.reshape(C, P).T.astype(np.int32).copy()
        xf = features[esrc]            # [EC, D] pre-permuted feature stream
        qs = rel_emb[erel]             # [EC, D] pre-permuted relation stream
        fslab = np.zeros((NLP, D), dtype=np.float32)
        fslab[:NL] = features[k * NL:(k + 1) * NL]
        per_core.append(dict(
            xf_in=xf, qs_in=qs, fslab_in=fslab, dstmod_in=dstmod,
            srcidx_in=srcidx,
        ))

    iota = np.tile(np.arange(P, dtype=np.float32), (P, 1))
    kbc = np.stack([np.tile(attn_k[l], (P, CH)) for l in range(DEPTH)])
    kbc = kbc.reshape(DEPTH, P, CH * D).astype(np.float32)
    for m in per_core:
        m["iota_in"] = iota
        m["kbc_in"] = kbc
    return per_core, CH


def kernel(**inputs):
    per_core, CH = _prepare(inputs)
    nc = _build(CH)
    results = run_bass_kernel_spmd(nc, per_core, core_ids=list(range(W))).results
    out = np.concatenate([r["out"][:NL] for r in results], axis=0)
    return out.astype(np.float32)


if __name__ == "__main__":
    import reference

    inputs = {k: np.asarray(v) for k, v in reference.setup_inputs().items()}
    got = kernel(**inputs)
    exp = np.asarray(reference.reference(**inputs))
    err = np.linalg.norm(got - exp) / np.linalg.norm(exp)
    print("Relative error:", err)
